# revision 1
# baseline (speedup 1.0000x reference)
"""BinarizedFCLayer forward on 8 trn2 NeuronCores.

    out = X @ sign(W).T      X: [8192, 2048] f32, W: [2048, 2048] f32
                             sign(w) = +1 if w >= 0 else -1

Strategy
--------
Data-parallel over the batch dim of X: core c computes rows
[c*1024, (c+1)*1024) of the output; W is replicated.

Per core (M=1024, K=2048, N=2048 -> 8.6 GFLOP(MAC), ~110 us at the 78.6 TF/s
16-bit TensorE peak; 24 MiB of input DMA, overlapped):
  * TensorE contracts over the partition dim, so both operands need K on
    partitions. The host passes X^T shards and W^T (pure layout prep).
  * X^T: SWDGE cast-DMA f32->fp16 into a resident tile. fp16 keeps 11
    mantissa bits -> output rel err ~2e-4 vs fp32 reference.
  * W^T: SWDGE cast-DMA f32->bf16 (bf16 keeps the f32 exponent range, so
    sign(bf16(w)) == sign(w) for every practical magnitude); DVE binarizes
    to exact +-1 fp16 (is_ge -> {1,0} -> *2-1) per (nn, kt-half) slice.
  * Warm-up matmuls run during the DMA prologue (emitted before everything
    so they queue first on PE) to hold the HAM clock gate at 8/8 (2.4 GHz).
  * PE: unit (nn, mq) = 2 PSUM banks, 32 matmuls of N=512 accumulating over
    16 k-tiles. PSUM->SBUF copies and 0.5 MiB output stores both ride
    ScalarE (ACT + its own HWDGE queue): in-engine ordering, short tail.

The walrus build here allows at most ONE sync wait per instruction, so a
post-pass splits any multi-wait instruction into single-wait NoOps on the
same engine placed immediately before it.
"""

import numpy as np

try:
    import concourse.bass as bass
except ImportError:  # harness may run from a bare directory
    import sys
    for p in ("/opt/trn_rl_repo", "/root/.axon_site/_ro/trn_rl_repo"):
        if p not in sys.path:
            sys.path.append(p)
    import concourse.bass as bass

import concourse.mybir as mybir
from concourse.tile import TileContext
from concourse.bass_utils import run_bass_kernel_spmd

P = 128
N_CORES = 8
M_FULL, K, N = 8192, 2048, 2048
M = M_FULL // N_CORES          # 1024 rows of X per core
KT = K // P                    # 16 k-tiles
KH = KT // 2                   # kt-half (DMA/binarize granularity)
MT = M // P                    # 8 m-tiles of 128
NCH, NW = 4, 512               # 4 n-chunks of 512 (one PSUM bank each)
MQ, MW = 4, 256                # m-quarters of 256 (2 m-tiles)
N_WARM = 330                   # dummy matmuls bridging preamble -> first data

f32 = mybir.dt.float32
f16 = mybir.dt.float16
bf16 = mybir.dt.bfloat16


def _split_multiwait_instructions(nc: bass.Bass) -> int:
    """walrus codegen rejects >1 sync wait per instruction. Hoist extra waits
    onto fresh single-wait NoOps on the same engine right before the
    offending instruction (same-engine sequential waits are equivalent)."""
    n_split = 0
    for fn in nc.m.functions:
        for blk in fn.blocks:
            out = []
            for inst in blk.instructions:
                si = inst.sync_info
                if si is not None and si.on_wait and len(si.on_wait) > 1:
                    waits = list(si.on_wait)
                    for j, w in enumerate(waits[:-1]):
                        nop = mybir.InstNoOp(
                            name=f"{inst.name}_wsplit{j}", ins=[], outs=[])
                        nop.engine = inst.engine
                        nop.sync_info = mybir.SyncInfo(
                            on_wait=[w], on_update=[])
                        out.append(nop)
                        n_split += 1
                    inst.sync_info = mybir.SyncInfo(
                        on_wait=[waits[-1]],
                        on_update=list(si.on_update or []))
                out.append(inst)
            blk.instructions[:] = out
    return n_split


def _build_nc() -> bass.Bass:
    nc = bass.Bass()
    xt = nc.declare_dram_parameter("xt", [K, M], f32, isOutput=False)
    wt = nc.declare_dram_parameter("wt", [K, N], f32, isOutput=False)
    out = nc.declare_dram_parameter("out", [M, N], f32, isOutput=True)

    xt3 = xt[:].rearrange("(kt p) m -> p kt m", p=P)    # [128, 16, 1024]
    wt3 = wt[:].rearrange("(kt p) n -> p kt n", p=P)    # [128, 16, 2048]
    out3 = out[:].rearrange("(mt p) n -> p mt n", p=P)  # [128, 8, 2048]

    with TileContext(nc) as tc:
        with (
            tc.tile_pool(name="resident", bufs=1) as res_pool,
            tc.tile_pool(name="wq", bufs=4) as wq_pool,
            tc.tile_pool(name="osb", bufs=4) as o_pool,
            tc.tile_pool(name="psum", bufs=8, space="PSUM") as p_pool,
            tc.tile_pool(name="warm", bufs=1) as warm_pool,
        ):
            # PE warm-up first: memset + dummy matmuls queue on PE before
            # anything else, so the HAM activity monitor un-throttles the
            # array while inputs stream in.
            wsrc = warm_pool.tile([P, P], f16, tag="wsrc", name="wsrc")
            nc.gpsimd.memset(wsrc[:], 0.0)
            wps = p_pool.tile([P, NW], f32, tag="ps", name="wps")
            for _ in range(N_WARM):
                nc.tensor.matmul(wps[:, :P], lhsT=wsrc[:], rhs=wsrc[:],
                                 start=True, stop=True)

            xq = res_pool.tile([P, KT, M], f16, tag="xq", name="xq")
            wraw = res_pool.tile([P, KT, N], bf16, tag="wraw", name="wraw")

            # Inputs: all SWDGE cast-DMAs into resident tiles (fresh
            # destinations -> zero-wait DMAs), ordered so the first unit's
            # data leads: W chunk-0 halves bracket X mq0's kt-halves.
            def wdma(nn, half):
                ks = slice(half * KH, (half + 1) * KH)
                ns = slice(nn * NW, (nn + 1) * NW)
                nc.gpsimd.dma_start(out=wraw[:, ks, ns], in_=wt3[:, ks, ns])

            def xdma(mq, half=None):
                ms = slice(mq * MW, (mq + 1) * MW)
                ks = slice(0, KT) if half is None else \
                    slice(half * KH, (half + 1) * KH)
                nc.gpsimd.dma_start(out=xq[:, ks, ms], in_=xt3[:, ks, ms])

            wdma(0, 0); wdma(0, 1); xdma(0)
            wdma(1, 0); wdma(1, 1); xdma(1)
            wdma(2, 0); wdma(2, 1); xdma(2)
            wdma(3, 0); wdma(3, 1); xdma(3)

            # Binarize each W (nn, kt-half) on DVE as slices land.
            wqs = []
            for nn in range(NCH):
                wq = wq_pool.tile([P, KT, NW], f16, tag="wq", name=f"wq{nn}")
                nsl = slice(nn * NW, (nn + 1) * NW)
                for h in range(2):
                    ks = slice(h * KH, (h + 1) * KH)
                    nc.vector.tensor_scalar(
                        wq[:, ks, :], wraw[:, ks, nsl], 0.0, None,
                        mybir.AluOpType.is_ge)
                    nc.vector.tensor_scalar(
                        wq[:, ks, :], wq[:, ks, :], 2.0, -1.0,
                        mybir.AluOpType.mult, mybir.AluOpType.add)
                wqs.append(wq)

            # PE: unit (nn, mq) = 2 psum banks; each kt-half is 16 matmuls
            # of N=512. Units ordered to match input arrival
            # (W0,X0,W1,X1,W2,X2,W3,X3): a unit is ready the moment its
            # W chunk and X quarter have landed.
            units = [(0, 0), (1, 0), (0, 1), (1, 1),
                     (2, 0), (2, 1), (0, 2), (1, 2), (2, 2),
                     (3, 0), (3, 1), (3, 2),
                     (0, 3), (1, 3), (2, 3), (3, 3)]
            half_order = [(nn, mq, h) for nn, mq in units for h in range(2)]
            unit_psums = {}
            for nn, mq, h in half_order:
                if h == 0:
                    unit_psums[(nn, mq)] = [
                        p_pool.tile([P, NW], f32, tag="ps",
                                    name=f"ps{nn}_{mq}_{i}")
                        for i in range(2)
                    ]
                psums = unit_psums[(nn, mq)]
                for kt in range(h * KH, (h + 1) * KH):
                    for mo in range(2):
                        mcol = mq * MW + mo * P
                        nc.tensor.matmul(
                            psums[mo][:],
                            lhsT=xq[:, kt, mcol:mcol + P],
                            rhs=wqs[nn][:, kt, :],
                            start=(kt == 0),
                            stop=(kt == KT - 1),
                        )
                if h == 1:
                    osb = o_pool.tile([P, 2, NW], f32, tag="osb",
                                      name=f"osb{nn}_{mq}")
                    for mo in range(2):
                        nc.scalar.activation(
                            out=osb[:, mo, :], in_=psums[mo][:],
                            func=mybir.ActivationFunctionType.Copy)
                    if (nn, mq) == (3, 3):
                        # split the very last store so the kernel-tail drain
                        # waits on a 0.25 MiB receipt, not 0.5 MiB
                        for mo in range(2):
                            nc.scalar.dma_start(
                                out=out3[:, mq * 2 + mo,
                                         nn * NW:(nn + 1) * NW],
                                in_=osb[:, mo, :])
                    else:
                        nc.scalar.dma_start(
                            out=out3[:, mq * 2:mq * 2 + 2,
                                     nn * NW:(nn + 1) * NW],
                            in_=osb[:])

    _split_multiwait_instructions(nc)
    return nc


_NC_CACHE = None


def _get_nc() -> bass.Bass:
    global _NC_CACHE
    if _NC_CACHE is None:
        _NC_CACHE = _build_nc()
    return _NC_CACHE


def _run(inputs: dict, trace: bool = False, **kw):
    X = np.asarray(inputs["X"], dtype=np.float32)
    W = np.asarray(inputs["W"], dtype=np.float32)
    assert X.shape == (M_FULL, K) and W.shape == (N, K)

    XT = np.ascontiguousarray(X.T)            # [K, M_FULL]
    WT = np.ascontiguousarray(W.T)            # [K, N]
    in_maps = [
        {"xt": np.ascontiguousarray(XT[:, c * M:(c + 1) * M]), "wt": WT}
        for c in range(N_CORES)
    ]
    res = run_bass_kernel_spmd(
        _get_nc(), in_maps, list(range(N_CORES)), trace=trace, **kw)
    out = np.concatenate([res.results[c]["out"] for c in range(N_CORES)],
                         axis=0)
    return out, res


def kernel(X: np.ndarray, W: np.ndarray) -> np.ndarray:
    out, _ = _run({"X": X, "W": W})
    return out



# revision 2
# speedup vs baseline: 1.0013x; 1.0013x over previous
"""BinarizedFCLayer forward on 8 trn2 NeuronCores.

    out = X @ sign(W).T      X: [8192, 2048] f32, W: [2048, 2048] f32
                             sign(w) = +1 if w >= 0 else -1

Strategy
--------
Data-parallel over the batch dim of X: core c computes rows
[c*1024, (c+1)*1024) of the output; W is replicated.

Per core (M=1024, K=2048, N=2048 -> 8.6 GFLOP(MAC), ~109 us at the 78.6 TF/s
16-bit TensorE peak; 24 MiB of f32 input DMA, overlapped):
  * All input DMA is plain f32 on the two HWDGE rings (W on SP/sync, X on
    ACT/scalar) in 1 MiB pieces with 8 KiB contiguous lines (host pre-packs
    both operands into [chunk, part, kt, free] layout). This replaces the
    baseline's SWDGE cast-DMAs, whose single-queue Q7 descriptor generation
    (~69 us serialized on GpSimd) capped input delivery and pushed the first
    real matmul to 28.6 us.
  * W is binarized on DVE straight from the staged f32 (is_ge -> {1,0} f16,
    then *2-1 -> exact +-1). X is cast f32->f16 on ACT (11 mantissa bits ->
    ~2e-4 output rel err).
  * PE: unit (nn, mq) = 2 PSUM banks, 32 matmuls of N=512 accumulating over
    16 k-tiles. Units run nn-minor ((0,0),(0,1)..(0,3),(1,0)..) so each
    4 MiB W chunk serves 4 consecutive units (27 us of PE work) while the
    next chunk streams in. DMA pieces are emitted in just-in-time order.
  * Warm-up matmuls bridge the DMA prologue and hold the HAM clock gate.
  * The last unit runs its two m-tiles serially so the final PSUM copy +
    0.25 MiB store overlap the other m-tile's matmuls (short kernel tail).

The walrus build here allows at most ONE sync wait per instruction, so a
post-pass splits any multi-wait instruction into single-wait NoOps on the
same engine placed immediately before it.
"""

import numpy as np

try:
    import concourse.bass as bass
except ImportError:  # harness may run from a bare directory
    import sys
    for p in ("/opt/trn_rl_repo", "/root/.axon_site/_ro/trn_rl_repo"):
        if p not in sys.path:
            sys.path.append(p)
    import concourse.bass as bass

import concourse.mybir as mybir
from concourse.tile import TileContext
from concourse.bass_utils import run_bass_kernel_spmd

P = 128
N_CORES = 8
M_FULL, K, N = 8192, 2048, 2048
M = M_FULL // N_CORES          # 1024 rows of X per core
KT = K // P                    # 16 k-tiles
NCH, NW = 4, 512               # 4 n-chunks of 512 (one PSUM bank each)
MQ, MW = 4, 256                # m-quarters of 256 (2 m-tiles)
KQ = 4                         # k-tiles per W DMA piece (1 MiB)
KH = 8                         # k-tiles per X DMA piece (1 MiB)
N_WARM = 88                    # dummy matmuls bridging preamble -> first data

f32 = mybir.dt.float32
f16 = mybir.dt.float16


def _split_multiwait_instructions(nc: bass.Bass) -> int:
    """walrus codegen rejects >1 sync wait per instruction. Hoist extra waits
    onto fresh single-wait NoOps on the same engine right before the
    offending instruction (same-engine sequential waits are equivalent)."""
    n_split = 0
    for fn in nc.m.functions:
        for blk in fn.blocks:
            out = []
            for inst in blk.instructions:
                si = inst.sync_info
                if si is not None and si.on_wait and len(si.on_wait) > 1:
                    waits = list(si.on_wait)
                    for j, w in enumerate(waits[:-1]):
                        nop = mybir.InstNoOp(
                            name=f"{inst.name}_wsplit{j}", ins=[], outs=[])
                        nop.engine = inst.engine
                        nop.sync_info = mybir.SyncInfo(
                            on_wait=[w], on_update=[])
                        out.append(nop)
                        n_split += 1
                    inst.sync_info = mybir.SyncInfo(
                        on_wait=[waits[-1]],
                        on_update=list(si.on_update or []))
                out.append(inst)
            blk.instructions[:] = out
    return n_split


def _build_nc() -> bass.Bass:
    nc = bass.Bass()
    # Host-packed layouts (see _run):
    #   xh[mq, p, kt, mw]: X^T quarter-major; piece (mq, kh) is 8 KiB/line.
    #   wh[nn, p, kt, nw]: W^T chunk-major; piece (nn, kq) is 8 KiB/line.
    xh = nc.declare_dram_parameter("xh", [MQ, P, KT, MW], f32, isOutput=False)
    wh = nc.declare_dram_parameter("wh", [NCH, P, KT, NW], f32, isOutput=False)
    out = nc.declare_dram_parameter("out", [M, N], f32, isOutput=True)

    out3 = out[:].rearrange("(mt p) n -> p mt n", p=P)  # [128, 8, 2048]

    with TileContext(nc) as tc:
        with (
            tc.tile_pool(name="resident", bufs=1) as res_pool,
            tc.tile_pool(name="wq", bufs=4) as wq_pool,
            tc.tile_pool(name="wstage", bufs=4) as ws_pool,
            tc.tile_pool(name="xstage", bufs=3) as xs_pool,
            tc.tile_pool(name="osb", bufs=4) as o_pool,
            tc.tile_pool(name="psum", bufs=8, space="PSUM") as p_pool,
            tc.tile_pool(name="warm", bufs=1) as warm_pool,
        ):
            # PE warm-up first: memset + dummy matmuls queue on PE before
            # anything else, so the HAM activity monitor un-throttles the
            # array while inputs stream in.
            wsrc = warm_pool.tile([P, P], f16, tag="wsrc", name="wsrc")
            nc.gpsimd.memset(wsrc[:], 0.0)
            wps = p_pool.tile([P, NW], f32, tag="ps", name="wps")
            for _ in range(N_WARM):
                nc.tensor.matmul(wps[:, :P], lhsT=wsrc[:], rhs=wsrc[:],
                                 start=True, stop=True)

            # Resident 16-bit operands.
            xq = res_pool.tile([P, MQ, KT, MW], f16, tag="xq", name="xq")
            wqs = [wq_pool.tile([P, KT, NW], f16, tag="wq", name=f"wq{nn}")
                   for nn in range(NCH)]

            # Input DMA: plain f32 on the two HWDGE rings, 1 MiB pieces,
            # emitted in just-in-time order for the nn-minor unit schedule.
            wstages = {}
            xstages = {}

            def wdma(nn, kq):
                t = ws_pool.tile([P, KQ, NW], f32, tag="ws",
                                 name=f"ws{nn}_{kq}")
                nc.sync.dma_start(
                    out=t[:], in_=wh[nn, :, kq * KQ:(kq + 1) * KQ, :])
                wstages[(nn, kq)] = t

            def xdma(mq, kh):
                t = xs_pool.tile([P, KH, MW], f32, tag="xs",
                                 name=f"xs{mq}_{kh}")
                nc.scalar.dma_start(
                    out=t[:], in_=xh[mq, :, kh * KH:(kh + 1) * KH, :])
                xstages[(mq, kh)] = t

            wdma(0, 0); xdma(0, 0); wdma(0, 1); xdma(0, 1)
            wdma(0, 2); xdma(1, 0); wdma(0, 3); xdma(1, 1)
            wdma(1, 0); xdma(2, 0); wdma(1, 1); xdma(2, 1)
            wdma(1, 2); xdma(3, 0); wdma(1, 3); xdma(3, 1)
            wdma(2, 0); wdma(2, 1); wdma(2, 2); wdma(2, 3)
            wdma(3, 0); wdma(3, 1); wdma(3, 2); wdma(3, 3)

            # Binarize each W piece on DVE as it lands (f32 -> exact +-1 f16).
            for nn in range(NCH):
                for kq in range(KQ):
                    ks = slice(kq * KQ, (kq + 1) * KQ)
                    nc.vector.tensor_scalar(
                        wqs[nn][:, ks, :], wstages[(nn, kq)][:], 0.0, None,
                        mybir.AluOpType.is_ge)
                    nc.vector.tensor_scalar(
                        wqs[nn][:, ks, :], wqs[nn][:, ks, :], 2.0, -1.0,
                        mybir.AluOpType.mult, mybir.AluOpType.add)

            # Cast each X piece f32 -> f16 on ACT.
            for mq in range(MQ):
                for kh in range(2):
                    ks = slice(kh * KH, (kh + 1) * KH)
                    nc.scalar.activation(
                        out=xq[:, mq, ks, :], in_=xstages[(mq, kh)][:],
                        func=mybir.ActivationFunctionType.Copy)

            # PE: unit (nn, mq) = 2 psum banks; 32 matmuls of N=512
            # accumulating over 16 k-tiles. nn-minor order matches the W
            # chunk stream; X quarters are cheap and prefetch easily.
            units = [(nn, mq) for nn in range(NCH) for mq in range(MQ)]
            for ui, (nn, mq) in enumerate(units):
                last = ui == len(units) - 1
                psums = [p_pool.tile([P, NW], f32, tag="ps",
                                     name=f"ps{nn}_{mq}_{i}")
                         for i in range(2)]
                osb = o_pool.tile([P, 2, NW], f32, tag="osb",
                                  name=f"osb{nn}_{mq}")

                def mm(kt, mo):
                    nc.tensor.matmul(
                        psums[mo][:],
                        lhsT=xq[:, mq, kt, mo * P:(mo + 1) * P],
                        rhs=wqs[nn][:, kt, :],
                        start=(kt == 0),
                        stop=(kt == KT - 1),
                    )

                def flush(mo):
                    nc.scalar.activation(
                        out=osb[:, mo, :], in_=psums[mo][:],
                        func=mybir.ActivationFunctionType.Copy)
                    nc.scalar.dma_start(
                        out=out3[:, mq * 2 + mo, nn * NW:(nn + 1) * NW],
                        in_=osb[:, mo, :])

                if last:
                    # m-serial: mt A's copy+store overlap mt B's matmuls.
                    for mo in range(2):
                        for kt in range(KT):
                            mm(kt, mo)
                        flush(mo)
                else:
                    for kt in range(KT):
                        for mo in range(2):
                            mm(kt, mo)
                    for mo in range(2):
                        flush(mo)

    _split_multiwait_instructions(nc)
    return nc


_NC_CACHE = None


def _get_nc() -> bass.Bass:
    global _NC_CACHE
    if _NC_CACHE is None:
        _NC_CACHE = _build_nc()
    return _NC_CACHE


def _pack_inputs(X: np.ndarray, W: np.ndarray):
    """Host-side layout prep (pure data movement, no value changes).

    xh[c]: [MQ, P, KT, MW] with xh[c][mq, p, kt, m] = X[c*M + mq*MW + m,
                                                        kt*P + p]
    wh:    [NCH, P, KT, NW] with wh[nn, p, kt, n] = W[nn*NW + n, kt*P + p]
    """
    XT = X.T.reshape(KT, P, N_CORES, MQ, MW)        # [kt, p, c, mq, mw]
    xh = np.ascontiguousarray(XT.transpose(2, 3, 1, 0, 4))  # [c, mq, p, kt, mw]
    WT = W.T.reshape(KT, P, NCH, NW)                # [kt, p, nn, nw]
    wh = np.ascontiguousarray(WT.transpose(2, 1, 0, 3))     # [nn, p, kt, nw]
    return xh, wh


def _run(inputs: dict, trace: bool = False, **kw):
    X = np.asarray(inputs["X"], dtype=np.float32)
    W = np.asarray(inputs["W"], dtype=np.float32)
    assert X.shape == (M_FULL, K) and W.shape == (N, K)

    xh, wh = _pack_inputs(X, W)
    in_maps = [{"xh": xh[c], "wh": wh} for c in range(N_CORES)]
    res = run_bass_kernel_spmd(
        _get_nc(), in_maps, list(range(N_CORES)), trace=trace, **kw)
    out = np.concatenate([res.results[c]["out"] for c in range(N_CORES)],
                         axis=0)
    return out, res


def kernel(X: np.ndarray, W: np.ndarray) -> np.ndarray:
    out, _ = _run({"X": X, "W": W})
    return out


# revision 4
# speedup vs baseline: 1.0074x; 1.0061x over previous
"""BinarizedFCLayer forward on 8 trn2 NeuronCores.

    out = X @ sign(W).T      X: [8192, 2048] f32, W: [2048, 2048] f32
                             sign(w) = +1 if w >= 0 else -1

Strategy
--------
Data-parallel over the batch dim of X: core c computes rows
[c*1024, (c+1)*1024) of the output; W is replicated.

Per core (M=1024, K=2048, N=2048 -> 8.6 GFLOP(MAC), ~109 us at the 78.6 TF/s
16-bit TensorE peak; 24 MiB of f32 input DMA, overlapped):
  * Strict engine separation (mixing input dma_starts into an engine stream
    that also has compute head-of-line-blocks that engine on staging-buffer
    waits):
      - GpSimd SWDGE: X cast-DMAs f32->f16 straight into the resident tile
        (no staging, no on-chip cast), k-major pieces covering all of M.
      - sync HWDGE ring: all W pieces, f32, 0.5 MiB each, just-in-time
        order; the staging-pool wait throttles only this ring.
      - ACT: PSUM->SBUF copies + output stores on the scalar HWDGE ring.
      - DVE: binarize W pieces f32 -> exact +-1 f16 (is_ge; *2-1).
    Host pre-packs both operands [chunk, part, kt, free] so every DMA line
    is 4-8 KiB contiguous.
  * PE schedule: for each W chunk nn (2048x512), run kt-outer across ALL
    8 PSUM banks (4 m-quarters x 2 m-tiles, N=512 each), accumulating 16
    k-tiles. Chunk 0 is consumed k-tile-by-k-tile as W/X stream in -- the
    DMA ramp overlaps 27 us of real matmuls instead of one unit's 6.8 us.
    Later chunks are fully resident when reached. The last chunk runs
    m-serial (unit-major) so the final PSUM copy + 0.25 MiB store overlap
    the remaining matmuls (short kernel tail).
  * Warm-up matmuls bridge the preamble and hold the HAM clock gate.

The walrus build here allows at most ONE sync wait per instruction, so a
post-pass splits any multi-wait instruction into single-wait NoOps on the
same engine placed immediately before it.
"""

import numpy as np

try:
    import concourse.bass as bass
except ImportError:  # harness may run from a bare directory
    import sys
    for p in ("/opt/trn_rl_repo", "/root/.axon_site/_ro/trn_rl_repo"):
        if p not in sys.path:
            sys.path.append(p)
    import concourse.bass as bass

import concourse.mybir as mybir
from concourse.tile import TileContext
from concourse.bass_utils import run_bass_kernel_spmd

P = 128
N_CORES = 8
M_FULL, K, N = 8192, 2048, 2048
M = M_FULL // N_CORES          # 1024 rows of X per core
KT = K // P                    # 16 k-tiles
NCH, NW = 4, 512               # 4 n-chunks of 512 (one PSUM bank each)
MQ, MW = 4, 256                # m-quarters of 256 (2 m-tiles)
WKP = 2                        # k-tiles per W DMA piece (0.5 MiB)
XKP = 4                        # k-tiles per X DMA piece (1 MiB)
N_WARM = 80                    # dummy matmuls bridging preamble -> first data

f32 = mybir.dt.float32
f16 = mybir.dt.float16


def _split_multiwait_instructions(nc: bass.Bass) -> int:
    """walrus codegen rejects >1 sync wait per instruction. Hoist extra waits
    onto fresh single-wait NoOps on the same engine right before the
    offending instruction (same-engine sequential waits are equivalent)."""
    n_split = 0
    for fn in nc.m.functions:
        for blk in fn.blocks:
            out = []
            for inst in blk.instructions:
                si = inst.sync_info
                if si is not None and si.on_wait and len(si.on_wait) > 1:
                    waits = list(si.on_wait)
                    for j, w in enumerate(waits[:-1]):
                        nop = mybir.InstNoOp(
                            name=f"{inst.name}_wsplit{j}", ins=[], outs=[])
                        nop.engine = inst.engine
                        nop.sync_info = mybir.SyncInfo(
                            on_wait=[w], on_update=[])
                        out.append(nop)
                        n_split += 1
                    inst.sync_info = mybir.SyncInfo(
                        on_wait=[waits[-1]],
                        on_update=list(si.on_update or []))
                out.append(inst)
            blk.instructions[:] = out
    return n_split


def _build_nc() -> bass.Bass:
    nc = bass.Bass()
    # Host-packed layouts (see _run):
    #   xh[mq, p, kt, mw]: X^T quarter-major; 4 KiB contiguous per (mq,p,kq).
    #   wh[nn, p, kt, nw]: W^T chunk-major; 4 KiB contiguous per (nn,p,kp).
    xh = nc.declare_dram_parameter("xh", [MQ, P, KT, MW], f32, isOutput=False)
    wh = nc.declare_dram_parameter("wh", [NCH, P, KT, NW], f32, isOutput=False)
    out = nc.declare_dram_parameter("out", [M, N], f32, isOutput=True)

    out3 = out[:].rearrange("(mt p) n -> p mt n", p=P)  # [128, 8, 2048]
    xh_r = xh[:].rearrange("mq p kt mw -> p mq kt mw")  # [128, 4, 16, 256]

    with TileContext(nc) as tc:
        with (
            tc.tile_pool(name="resident", bufs=1) as res_pool,
            tc.tile_pool(name="wq", bufs=4) as wq_pool,
            tc.tile_pool(name="wstage", bufs=6) as ws_pool,
            tc.tile_pool(name="osb", bufs=6) as o_pool,
            tc.tile_pool(name="psum", bufs=8, space="PSUM") as p_pool,
            tc.tile_pool(name="warm", bufs=1) as warm_pool,
        ):
            # PE warm-up first: memset + dummy matmuls queue on PE before
            # anything else, so the HAM activity monitor un-throttles the
            # array while inputs stream in.
            wsrc = warm_pool.tile([P, P], f16, tag="wsrc", name="wsrc")
            nc.vector.memset(wsrc[:], 0.0)
            wps = p_pool.tile([P, NW], f32, tag="ps", name="wps")
            for _ in range(N_WARM):
                nc.tensor.matmul(wps[:, :P], lhsT=wsrc[:], rhs=wsrc[:],
                                 start=True, stop=True)

            # Resident 16-bit operands.
            xq = res_pool.tile([P, MQ, KT, MW], f16, tag="xq", name="xq")
            wqs = [wq_pool.tile([P, KT, NW], f16, tag="wq", name=f"wq{nn}")
                   for nn in range(NCH)]

            # X: SWDGE cast-DMA f32->f16, k-major pieces covering all of M,
            # directly into the resident tile (no staging -> never throttled).
            for kp in range(KT // XKP):
                ks = slice(kp * XKP, (kp + 1) * XKP)
                nc.gpsimd.dma_start(out=xq[:, :, ks, :], in_=xh_r[:, :, ks, :])

            # W: plain f32 pieces on the sync HWDGE ring, just-in-time order;
            # DVE binarizes each piece as it lands.
            for nn in range(NCH):
                for kp in range(KT // WKP):
                    ks = slice(kp * WKP, (kp + 1) * WKP)
                    t = ws_pool.tile([P, WKP, NW], f32, tag="ws",
                                     name=f"ws{nn}_{kp}")
                    nc.sync.dma_start(out=t[:], in_=wh[nn, :, ks, :])
                    nc.vector.tensor_scalar(
                        wqs[nn][:, ks, :], t[:], 0.0, None,
                        mybir.AluOpType.is_ge)
                    nc.vector.tensor_scalar(
                        wqs[nn][:, ks, :], wqs[nn][:, ks, :], 2.0, -1.0,
                        mybir.AluOpType.mult, mybir.AluOpType.add)

            def flush(nn, mq, mo, psum):
                nc.scalar.activation(
                    out=osbs[(mq, mo)][:], in_=psum[:],
                    func=mybir.ActivationFunctionType.Copy)
                nc.scalar.dma_start(
                    out=out3[:, mq * 2 + mo, nn * NW:(nn + 1) * NW],
                    in_=osbs[(mq, mo)][:])

            # PE: per W chunk, kt-outer across all 8 PSUM banks (4 mq x 2 mo)
            # -- chunk 0 streams k-tile-by-k-tile as the inputs land. The
            # last chunk runs m-serial so its stores overlap remaining MMs.
            for nn in range(NCH):
                psums = {(mq, mo): p_pool.tile([P, NW], f32, tag="ps",
                                               name=f"ps{nn}_{mq}_{mo}")
                         for mq in range(MQ) for mo in range(2)}
                osbs = {(mq, mo): o_pool.tile([P, NW], f32, tag="osb",
                                              name=f"osb{nn}_{mq}_{mo}")
                        for mq in range(MQ) for mo in range(2)}

                def mm(kt, mq, mo):
                    nc.tensor.matmul(
                        psums[(mq, mo)][:],
                        lhsT=xq[:, mq, kt, mo * P:(mo + 1) * P],
                        rhs=wqs[nn][:, kt, :],
                        start=(kt == 0),
                        stop=(kt == KT - 1),
                    )

                if nn < NCH - 1:
                    for kt in range(KT):
                        for mq in range(MQ):
                            for mo in range(2):
                                mm(kt, mq, mo)
                    for mq in range(MQ):
                        for mo in range(2):
                            flush(nn, mq, mo, psums[(mq, mo)])
                else:
                    for mq in range(MQ):
                        for mo in range(2):
                            for kt in range(KT):
                                mm(kt, mq, mo)
                            flush(nn, mq, mo, psums[(mq, mo)])

    _split_multiwait_instructions(nc)
    return nc


_NC_CACHE = None


def _get_nc() -> bass.Bass:
    global _NC_CACHE
    if _NC_CACHE is None:
        _NC_CACHE = _build_nc()
    return _NC_CACHE


def _pack_inputs(X: np.ndarray, W: np.ndarray):
    """Host-side layout prep (pure data movement, no value changes).

    xh[c]: [MQ, P, KT, MW] with xh[c][mq, p, kt, m] = X[c*M + mq*MW + m,
                                                        kt*P + p]
    wh:    [NCH, P, KT, NW] with wh[nn, p, kt, n] = W[nn*NW + n, kt*P + p]
    """
    XT = X.T.reshape(KT, P, N_CORES, MQ, MW)        # [kt, p, c, mq, mw]
    xh = np.ascontiguousarray(XT.transpose(2, 3, 1, 0, 4))  # [c, mq, p, kt, mw]
    WT = W.T.reshape(KT, P, NCH, NW)                # [kt, p, nn, nw]
    wh = np.ascontiguousarray(WT.transpose(2, 1, 0, 3))     # [nn, p, kt, nw]
    return xh, wh


def _run(inputs: dict, trace: bool = False, **kw):
    X = np.asarray(inputs["X"], dtype=np.float32)
    W = np.asarray(inputs["W"], dtype=np.float32)
    assert X.shape == (M_FULL, K) and W.shape == (N, K)

    xh, wh = _pack_inputs(X, W)
    in_maps = [{"xh": xh[c], "wh": wh} for c in range(N_CORES)]
    res = run_bass_kernel_spmd(
        _get_nc(), in_maps, list(range(N_CORES)), trace=trace, **kw)
    out = np.concatenate([res.results[c]["out"] for c in range(N_CORES)],
                         axis=0)
    return out, res


def kernel(X: np.ndarray, W: np.ndarray) -> np.ndarray:
    out, _ = _run({"X": X, "W": W})
    return out
